# revision 25
# baseline (speedup 1.0000x reference)
"""Trainium2 Bass kernel for CombinedLoss (cross-entropy + neural-collapse margin).

loss = mean_b( logsumexp(outputs[b]) - outputs[b, label_b] )
     + 0.1 * mean_b( relu(5 - ||features[b] - means[label_b]||) )

Strategy (8 NeuronCores, data-parallel over the batch):
  - Each core gets 2048 rows of outputs/features/labels; the [1000, 512]
    class-mean table is replicated in HBM (bf16) and read with SWDGE
    dma_gather: g_all[p, t, :] = means[label[t*128+p]].
  - ACT only ever runs Exp in the loop (activation-table reloads cost ~1.3us,
    so Ln/Sqrt/Relu are batched over [128, NT] columns at the end).
  - x[b, label_b] via one fused DVE op per tile: (iota == label) * x with
    sum-accumulate (scalar_tensor_tensor); it enters the loss only through
    its batch SUM, so no per-row log/extraction tail is needed.
  - Two phases so engine queues match data arrival: phase A (x DMAs + ACT exp
    + DVE extract) runs while the mean-gather descriptors generate; phase B
    (f DMAs + batched diff + square-accumulate, split DVE/ACT) consumes the
    gathered means.
  - Per-core partial sums [128, 2] are reduced on host (all-reduce of scalars).
"""

import os
import sys

import numpy as np

for _p in ("/opt/trn_rl_repo", "/opt/pypackages"):
    if os.path.isdir(_p) and _p not in sys.path:
        sys.path.insert(0, _p)

import concourse.bacc as bacc
import concourse.tile as tile
from concourse import bass_utils, mybir

B, C, D = 16384, 1000, 512
NCORES = 8
BC = B // NCORES  # rows per core
P = 128  # partitions
NT = BC // P  # tiles per core
TPD = 4  # tiles per DMA batch / diff batch
NB = NT // TPD  # blocks
EPS = 5.0
CLS_W, COL_W = 1.0, 0.1

_CACHE = {}


def _build():
    f32 = mybir.dt.float32
    bf16 = mybir.dt.bfloat16
    i16 = mybir.dt.int16
    u16 = mybir.dt.uint16
    f16 = mybir.dt.float16
    AF = mybir.ActivationFunctionType
    ALU = mybir.AluOpType

    nc = bacc.Bacc(
        "TRN2",
        target_bir_lowering=False,
        debug=False,
        enable_asserts=False,
        num_devices=NCORES,
    )
    xs = nc.dram_tensor("xs", [BC, C], bf16, kind="ExternalInput").ap()
    fs = nc.dram_tensor("fs", [BC, D], bf16, kind="ExternalInput").ap()
    mb = nc.dram_tensor("mb", [C, D], bf16, kind="ExternalInput").ap()
    li = nc.dram_tensor("li", [P, P], i16, kind="ExternalInput").ap()
    lp = nc.dram_tensor("lp", [P, NT], f16, kind="ExternalInput").ap()
    io = nc.dram_tensor("io", [P, C], f16, kind="ExternalInput").ap()
    cc = nc.dram_tensor("cc", [P, 2], f32, kind="ExternalInput").ap()
    po = nc.dram_tensor("po", [P, 2], f32, kind="ExternalOutput").ap()

    xs4 = xs.rearrange("(n t p) c -> n p t c", p=P, t=TPD)
    fs4 = fs.rearrange("(n t p) d -> n p t d", p=P, t=TPD)

    from contextlib import ExitStack

    with tile.TileContext(nc) as tc, ExitStack() as ctx:
        persist = ctx.enter_context(tc.tile_pool(name="persist", bufs=1))
        xpool = ctx.enter_context(tc.tile_pool(name="xpool", bufs=3))
        fpool = ctx.enter_context(tc.tile_pool(name="fpool", bufs=3))
        scratch = ctx.enter_context(tc.tile_pool(name="scratch", bufs=3))

        x4_first = xpool.tile([P, TPD, C], bf16, tag="x4")
        nc.sync.dma_start(out=x4_first, in_=xs4[0])
        li_sb = persist.tile([P, P], i16)
        nc.sync.dma_start(out=li_sb, in_=li)
        lp_sb = persist.tile([P, NT], f16)
        nc.sync.dma_start(out=lp_sb, in_=lp)
        iota_c = persist.tile([P, C], f16)
        nc.sync.dma_start(out=iota_c, in_=io)
        cc_sb = persist.tile([P, 2], f32)
        nc.sync.dma_start(out=cc_sb, in_=cc)
        eps_col = cc_sb[:, 0:1]
        zero_col = cc_sb[:, 1:2]

        # Gathered class means for all 2048 rows:
        # g_all[p, t, :] = means[label[t*128+p]]
        g_all = persist.tile([P, NT, D], bf16)
        NG = int(os.environ.get("K_NGCHUNK", "4"))
        rpc = BC // NG
        tpc = NT // NG
        scols = rpc // 16
        for g in range(NG):
            nc.gpsimd.dma_gather(
                out_ap=g_all[:, g * tpc : (g + 1) * tpc, :],
                in_ap=mb,
                idxs_ap=li_sb[:, g * scols : (g + 1) * scols],
                num_idxs=rpc,
                num_idxs_reg=rpc,
                elem_size=D,
            )

        sumexp_cols = persist.tile([P, NT], f32)
        xlab_cols = persist.tile([P, NT], f32)
        dsq_cols = persist.tile([P, NT], f32)

        # Phase A: cross-entropy over all tiles (x DMAs + ACT exp + DVE extract)
        for n in range(NB):
            if n == 0:
                x4 = x4_first
            else:
                x4 = xpool.tile([P, TPD, C], bf16, tag="x4")
                nc.sync.dma_start(out=x4, in_=xs4[n])
            for j in range(TPD):
                t = n * TPD + j
                e_scr = scratch.tile([P, C], bf16, tag="e_scr")
                nc.scalar.activation(
                    out=e_scr,
                    in_=x4[:, j, :],
                    func=AF.Exp,
                    bias=zero_col,
                    accum_out=sumexp_cols[:, t : t + 1],
                )
                m_scr = scratch.tile([P, C], f16, tag="m_scr")
                nc.vector.scalar_tensor_tensor(
                    out=m_scr,
                    in0=iota_c,
                    scalar=lp_sb[:, t : t + 1],
                    in1=x4[:, j, :],
                    op0=ALU.is_equal,
                    op1=ALU.mult,
                    accum_out=xlab_cols[:, t : t + 1],
                )

        # Phase B: collapse margin (f DMAs + diff + fused square-accumulate)
        for n in range(NB):
            f4 = fpool.tile([P, TPD, D], bf16, tag="f4")
            nc.sync.dma_start(out=f4, in_=fs4[n])
            diff4 = scratch.tile([P, TPD, D], bf16, tag="diff4")
            nc.vector.tensor_tensor(
                out=diff4,
                in0=f4,
                in1=g_all[:, n * TPD : (n + 1) * TPD, :],
                op=ALU.subtract,
            )
            for j in range(TPD):
                t = n * TPD + j
                sq_scr = scratch.tile([P, D], bf16, tag="sq_scr")
                if n >= NB // 2:
                    # ACT is idle once the exps are done; give it the back half
                    nc.scalar.activation(
                        out=sq_scr,
                        in_=diff4[:, j, :],
                        func=AF.Square,
                        bias=zero_col,
                        accum_out=dsq_cols[:, t : t + 1],
                    )
                else:
                    nc.vector.scalar_tensor_tensor(
                        out=sq_scr,
                        in0=diff4[:, j, :],
                        scalar=1.0,
                        op0=ALU.mult,
                        in1=diff4[:, j, :],
                        op1=ALU.mult,
                        accum_out=dsq_cols[:, t : t + 1],
                    )

        # Batched tail: one Ln, one Sqrt, one Relu over [P, NT] columns
        lse_cols = persist.tile([P, NT], f32)
        nc.scalar.activation(out=lse_cols, in_=sumexp_cols, func=AF.Ln, bias=zero_col)
        dist_cols = persist.tile([P, NT], f32)
        nc.scalar.activation(out=dist_cols, in_=dsq_cols, func=AF.Sqrt, bias=zero_col)
        # -relu(eps - dist) == min(dist - eps, 0); host negates the sum
        relu_cols = persist.tile([P, NT], f32)
        nc.vector.tensor_scalar(
            out=relu_cols,
            in0=dist_cols,
            scalar1=EPS,
            scalar2=0.0,
            op0=ALU.subtract,
            op1=ALU.min,
        )

        # partials[:,0] = sum_t lse - sum_n xlab ; partials[:,1] = sum_t relu
        lse_red = persist.tile([P, 1], f32)
        nc.vector.tensor_reduce(
            out=lse_red, in_=lse_cols, axis=mybir.AxisListType.X, op=ALU.add
        )
        xlab_red = persist.tile([P, 1], f32)
        nc.vector.tensor_reduce(
            out=xlab_red, in_=xlab_cols, axis=mybir.AxisListType.X, op=ALU.add
        )
        partials = persist.tile([P, 2], f32)
        nc.vector.tensor_tensor(
            out=partials[:, 0:1], in0=lse_red, in1=xlab_red, op=ALU.subtract
        )
        nc.vector.tensor_reduce(
            out=partials[:, 1:2], in_=relu_cols, axis=mybir.AxisListType.X, op=ALU.add
        )
        nc.sync.dma_start(out=po, in_=partials)

    nc.compile()
    return nc


def get_nc():
    if "nc" not in _CACHE:
        _CACHE["nc"] = _build()
    return _CACHE["nc"]


def make_in_maps(outputs, features, target_means, target_labels):
    bf16np = mybir.dt.np(mybir.dt.bfloat16)
    outputs = np.ascontiguousarray(np.asarray(outputs, dtype=np.float32).astype(bf16np))
    features = np.ascontiguousarray(np.asarray(features, dtype=np.float32).astype(bf16np))
    means = np.asarray(target_means, dtype=np.float32)
    labels = np.asarray(target_labels).astype(np.int64)

    means_bf = np.ascontiguousarray(means.astype(bf16np))
    iota = np.ascontiguousarray(
        np.broadcast_to(np.arange(C, dtype=np.float16)[None, :], (P, C))
    )
    consts = np.ascontiguousarray(
        np.broadcast_to(np.array([EPS, 0.0], dtype=np.float32)[None, :], (P, 2))
    )

    in_maps = []
    for k in range(NCORES):
        sl = slice(k * BC, (k + 1) * BC)
        lab = labels[sl]
        # dma_gather reads indices wrapped (s p) over the first 16 partitions,
        # replicated to all 8 gpsimd cores.
        base = lab.reshape(BC // 16, 16).T  # [16, 128]; base[r, s] = lab[s*16+r]
        li = np.ascontiguousarray(np.tile(base, (8, 1)).astype(np.int16))
        lp = np.ascontiguousarray(lab.reshape(NT, P).T.astype(np.float16))
        in_maps.append(
            {
                "xs": outputs[sl],
                "fs": features[sl],
                "mb": means_bf,
                "li": li,
                "lp": lp,
                "io": iota,
                "cc": consts,
            }
        )
    return in_maps


def run(trace=False, **inputs):
    nc = get_nc()
    in_maps = make_in_maps(
        inputs["outputs"],
        inputs["features"],
        inputs["target_means"],
        inputs["target_labels"],
    )
    last_err = None
    for _attempt in range(3):
        try:
            res = bass_utils.run_bass_kernel_spmd(
                nc, in_maps, core_ids=list(range(NCORES)), trace=trace
            )
            break
        except Exception as e:  # device occasionally needs a retry after reset
            last_err = e
    else:
        raise last_err
    ce_sum = 0.0
    relu_sum = 0.0  # device returns -relu sums
    for r in res.results:
        p = np.asarray(r["po"], dtype=np.float64)
        ce_sum += float(p[:, 0].sum())
        relu_sum -= float(p[:, 1].sum())
    loss = (CLS_W * ce_sum + COL_W * relu_sum) / B
    return np.asarray(loss, dtype=np.float32), res


def kernel(**inputs):
    loss, _ = run(trace=False, **inputs)
    return loss
